# revision 9
# baseline (speedup 1.0000x reference)
"""Trainium2 Bass kernel for nn_Contraction_627065225897 (MACE-style symmetric
contraction with per-element (MoE-routed) weights).

Math (per atom n with element e = sorted_indices[n], channel f):
  out[n,f] = sum_p W3[e,p,f] * T3[n,f,p]  +  sum_q W2[e,q,f] * T2[n,f,q]
           + W1[e,0,f] * T1[n,f]
  T3[n,f,p]   = sum_{m1,m2,k} U3[p,m1,m2,k] x_m1 x_m2 x_k
  T2[n,f,q]   = sum_{a,b}     U2[q,a,b]     x_a  x_b
  T1[n,f]     = sum_l         U1[0,l]       x_l
(Equivalent to the reference's Horner evaluation; verified in fp64/fp32.)

Device strategy (per core, 16 atoms, f=128 on partitions):
  - xx[f, (m2,k)] outer products via one DVE op (stride-0 APs)
  - PE transposes xx -> xxT (contraction dim on partitions)
  - PE matmul: T3h[f, (p,m1)+(q)] = xxT.T @ U3m  (moving operand shared)
  - DVE fused multiply-reduce collapses (p,m1) with W3*x weights
Sharding: data-parallel over atoms, 16 atoms/core on 8 cores; per-element
weights are gathered host-side by sorted_indices (routing) and replicated.
"""

import os
import sys
from contextlib import ExitStack

import numpy as np

if "/opt/trn_rl_repo" not in sys.path:
    sys.path.insert(0, "/opt/trn_rl_repo")

B, F, L = 128, 128, 16
E = 10
P3, P2, P1 = 23, 4, 1
NCORES = 8
BS = B // NCORES  # atoms per core = 16
NPM = P3 * L  # 368 = (p, m1) columns, p-major is NOT used; m1-minor within p
NCOL = NPM + P2  # 372 total moving columns (cubic + quadratic)

_CACHE = {}


def _build_program(use_f32r: bool):
    import concourse.bass as bass
    import concourse.mybir as mybir
    import concourse.tile as tile
    from concourse import bacc

    dt = mybir.dt.float32
    nc = bacc.Bacc("TRN2", target_bir_lowering=False, debug=False)

    xs_d = nc.dram_tensor("xs", [128, BS * L], dt, kind="ExternalInput")
    mov_d = nc.dram_tensor("mov", [2, 128, NCOL], dt, kind="ExternalInput")
    cw3_d = nc.dram_tensor("cw3", [128, BS * P3], dt, kind="ExternalInput")
    w2s_d = nc.dram_tensor("w2s", [128, BS * P2], dt, kind="ExternalInput")
    w1s_d = nc.dram_tensor("w1s", [128, BS], dt, kind="ExternalInput")
    u1b_d = nc.dram_tensor("u1b", [128, L], dt, kind="ExternalInput")
    id_d = nc.dram_tensor("ident", [128, 128], dt, kind="ExternalInput")
    out_d = nc.dram_tensor("outT", [128, BS], dt, kind="ExternalOutput")

    mult = mybir.AluOpType.mult
    add = mybir.AluOpType.add

    with tile.TileContext(nc) as tc, ExitStack() as ctx:
        const = ctx.enter_context(tc.tile_pool(name="const", bufs=1))
        work = ctx.enter_context(tc.tile_pool(name="work", bufs=4))
        ps_xx = ctx.enter_context(
            tc.tile_pool(name="ps_xx", bufs=3, space=bass.MemorySpace.PSUM)
        )
        ps_t3 = ctx.enter_context(
            tc.tile_pool(name="ps_t3", bufs=5, space=bass.MemorySpace.PSUM)
        )

        XS = const.tile([128, BS * L], dt)
        nc.sync.dma_start(XS[:], xs_d.ap())
        MOV0 = const.tile([128, NCOL], dt)
        nc.sync.dma_start(MOV0[:], mov_d.ap()[0])
        MOV1 = const.tile([128, NCOL], dt)
        nc.sync.dma_start(MOV1[:], mov_d.ap()[1])
        CW3 = const.tile([128, BS * P3], dt)
        nc.sync.dma_start(CW3[:], cw3_d.ap())
        W2S = const.tile([128, BS * P2], dt)
        nc.sync.dma_start(W2S[:], w2s_d.ap())
        W1S = const.tile([128, BS], dt)
        nc.sync.dma_start(W1S[:], w1s_d.ap())
        U1B = const.tile([128, L], dt)
        nc.sync.dma_start(U1B[:], u1b_d.ap())
        IDENT = const.tile([128, 128], dt)
        nc.sync.dma_start(IDENT[:], id_d.ap())

        # xx[f, (n, m2, k)] = x[f, n, m2] * x[f, n, k] — one DVE op
        XX = const.tile([128, BS * L * L], dt)
        xs3 = XS[:].rearrange("p (n a) -> p n a", n=BS)
        in0 = xs3.unsqueeze(3).broadcast_to((128, BS, L, L))  # x_m2
        in1 = xs3.unsqueeze(2).broadcast_to((128, BS, L, L))  # x_k
        xxv = XX[:].rearrange("p (n a b) -> p n a b", n=BS, a=L)
        nc.vector.tensor_tensor(xxv, in0, in1, op=mult)

        OUT = const.tile([128, BS], dt)
        ACC = const.tile([128, BS * 3], dt)  # per atom: [lin, quad, cubic]
        XXS = const.tile([128, BS * 256], dt)  # all atoms' transposed xx
        W3X = const.tile([128, BS * NPM], dt)  # all atoms' W3*x weights

        maybe_r = (
            (lambda ap: ap.bitcast(mybir.dt.float32r)) if use_f32r else (lambda ap: ap)
        )

        # Phase A: PE transposes + ACT evacuation; GPSIMD builds w3x in parallel
        for n in range(BS):
            xxp = ps_xx.tile([128, 256], dt, tag="xxp")
            nc.tensor.transpose(
                xxp[:, 0:128], XX[:, n * 256 : n * 256 + 128], IDENT[:]
            )
            nc.tensor.transpose(
                xxp[:, 128:256], XX[:, n * 256 + 128 : n * 256 + 256], IDENT[:]
            )
            nc.scalar.copy(XXS[:, n * 256 : (n + 1) * 256], xxp[:])

            # w3x[f, (p, m1)] = W3[e_n, p, f] * x[f, m1]  (GPSIMD, off DVE)
            c0 = CW3[:, n * P3 : (n + 1) * P3].unsqueeze(2).broadcast_to((128, P3, L))
            x0 = XS[:, n * L : (n + 1) * L].unsqueeze(1).broadcast_to((128, P3, L))
            nc.gpsimd.tensor_tensor(
                W3X[:, n * NPM : (n + 1) * NPM].rearrange(
                    "p (a b) -> p a b", a=P3
                ),
                c0,
                x0,
                op=mult,
            )
            # linear: sum_l x_l * W1 * U1_l (DVE, independent of PE)
            sc16 = work.tile([128, L], dt, tag="sc16")
            nc.vector.scalar_tensor_tensor(
                out=sc16[:],
                in0=XS[:, n * L : (n + 1) * L],
                scalar=W1S[:, n : n + 1],
                in1=U1B[:],
                op0=mult,
                op1=mult,
                accum_out=ACC[:, 3 * n : 3 * n + 1],
            )

        # Phase B: PE matmuls + DVE reductions chasing them
        for n in range(BS):
            t3 = ps_t3.tile([128, NCOL], dt, tag="t3")
            nc.tensor.matmul(
                t3[:],
                maybe_r(XXS[:, n * 256 : n * 256 + 128]),
                maybe_r(MOV0[:]),
                start=True,
                stop=False,
            )
            nc.tensor.matmul(
                t3[:],
                maybe_r(XXS[:, n * 256 + 128 : n * 256 + 256]),
                maybe_r(MOV1[:]),
                start=False,
                stop=True,
            )
            # quadratic: sum_q T2_q * W2_q
            sc4 = work.tile([128, P2], dt, tag="sc4")
            nc.vector.scalar_tensor_tensor(
                out=sc4[:],
                in0=t3[:, NPM:NCOL],
                scalar=1.0,
                in1=W2S[:, n * P2 : (n + 1) * P2],
                op0=mult,
                op1=mult,
                accum_out=ACC[:, 3 * n + 1 : 3 * n + 2],
            )
            # cubic: sum_{p,m1} T3 * w3x
            sc368 = work.tile([128, NPM], dt, tag="sc368")
            nc.vector.scalar_tensor_tensor(
                out=sc368[:],
                in0=t3[:, 0:NPM],
                scalar=1.0,
                in1=W3X[:, n * NPM : (n + 1) * NPM],
                op0=mult,
                op1=mult,
                accum_out=ACC[:, 3 * n + 2 : 3 * n + 3],
            )

        nc.vector.tensor_reduce(
            OUT[:],
            ACC[:].rearrange("p (n c) -> p n c", n=BS),
            axis=mybir.AxisListType.X,
            op=add,
        )
        nc.sync.dma_start(out_d.ap(), OUT[:])

    nc.compile()
    return nc


def _host_prep(x, sorted_indices, weights_max, w2, w1, U3, U2, U1):
    """Build per-core input maps (pure layout/gather work)."""
    x = np.ascontiguousarray(x, dtype=np.float32)
    si = np.asarray(sorted_indices).astype(np.int64)
    W3 = np.asarray(weights_max, dtype=np.float32)
    W2 = np.asarray(w2, dtype=np.float32)
    W1 = np.asarray(w1, dtype=np.float32)
    U3 = np.asarray(U3, dtype=np.float32)
    U2 = np.asarray(U2, dtype=np.float32)
    U1 = np.asarray(U1, dtype=np.float32)

    U3r = U3.reshape(P3, L, L, L)  # [p, m1, m2, k]
    U3m = np.ascontiguousarray(U3r.transpose(2, 3, 0, 1).reshape(L * L, P3 * L))
    U2m = np.ascontiguousarray(U2.reshape(P2, L * L).T)  # [(a,b), q]
    mov = np.concatenate([U3m, U2m], axis=1).reshape(2, 128, NCOL)
    mov = np.ascontiguousarray(mov)

    u1b = np.ascontiguousarray(np.tile(U1.reshape(1, L), (128, 1)))
    ident = np.eye(128, dtype=np.float32)

    in_maps = []
    for c in range(NCORES):
        sl = slice(c * BS, (c + 1) * BS)
        sic = si[sl]
        xs = np.ascontiguousarray(x[sl].transpose(1, 0, 2).reshape(128, BS * L))
        cw3 = np.ascontiguousarray(
            W3[sic].transpose(2, 0, 1).reshape(128, BS * P3)
        )
        w2s = np.ascontiguousarray(W2[sic].transpose(2, 0, 1).reshape(128, BS * P2))
        w1s = np.ascontiguousarray(W1[sic][:, 0, :].T)
        in_maps.append(
            {
                "xs": xs,
                "mov": mov,
                "cw3": cw3,
                "w2s": w2s,
                "w1s": w1s,
                "u1b": u1b,
                "ident": ident,
            }
        )
    return in_maps


def _get_nc():
    use_f32r = os.environ.get("KERNEL_F32R", "0") == "1"
    key = ("nc", use_f32r)
    if key not in _CACHE:
        _CACHE[key] = _build_program(use_f32r)
    return _CACHE[key]


def kernel(
    x,
    bincount,
    sorted_indices,
    weights_max,
    w2,
    w1,
    U3,
    U2,
    U1,
    _trace=False,
):
    from concourse.bass_utils import run_bass_kernel_spmd

    nc = _get_nc()
    in_maps = _host_prep(x, sorted_indices, weights_max, w2, w1, U3, U2, U1)
    res = run_bass_kernel_spmd(
        nc, in_maps, core_ids=list(range(NCORES)), trace=_trace
    )
    outs = [res.results[c]["outT"] for c in range(NCORES)]  # each [128f, 16n]
    full = np.concatenate([o.T for o in outs], axis=0)  # [128, 128]
    out = np.ascontiguousarray(full, dtype=np.float32)
    if _trace:
        return out, res
    return out


# revision 10
# speedup vs baseline: 1.0011x; 1.0011x over previous
"""Trainium2 Bass kernel for nn_Contraction_627065225897 (MACE-style symmetric
contraction with per-element (MoE-routed) weights).

Math (per atom n with element e = sorted_indices[n], channel f):
  out[n,f] = sum_p W3[e,p,f] * T3[n,f,p]  +  sum_q W2[e,q,f] * T2[n,f,q]
           + W1[e,0,f] * T1[n,f]
  T3[n,f,p]   = sum_{m1,m2,k} U3[p,m1,m2,k] x_m1 x_m2 x_k
  T2[n,f,q]   = sum_{a,b}     U2[q,a,b]     x_a  x_b
  T1[n,f]     = sum_l         U1[0,l]       x_l
(Equivalent to the reference's Horner evaluation; verified in fp64/fp32.)

Device strategy (per core, 16 atoms, f=128 on partitions):
  - xx[f, (m2,k)] outer products via one DVE op (stride-0 APs)
  - PE transposes xx -> xxT (contraction dim on partitions)
  - PE matmul: T3h[f, (p,m1)+(q)] = xxT.T @ U3m  (moving operand shared)
  - DVE fused multiply-reduce collapses (p,m1) with W3*x weights
Sharding: data-parallel over atoms, 16 atoms/core on 8 cores; per-element
weights are gathered host-side by sorted_indices (routing) and replicated.
"""

import os
import sys
from contextlib import ExitStack

import numpy as np

if "/opt/trn_rl_repo" not in sys.path:
    sys.path.insert(0, "/opt/trn_rl_repo")

B, F, L = 128, 128, 16
E = 10
P3, P2, P1 = 23, 4, 1
NCORES = 8
BS = B // NCORES  # atoms per core = 16
NPM = P3 * L  # 368 = (p, m1) columns, p-major is NOT used; m1-minor within p
NCOL = NPM + P2  # 372 total moving columns (cubic + quadratic)

_CACHE = {}


def _build_program(use_f32r: bool):
    import concourse.bass as bass
    import concourse.mybir as mybir
    import concourse.tile as tile
    from concourse import bacc

    dt = mybir.dt.float32
    nc = bacc.Bacc("TRN2", target_bir_lowering=False, debug=False)

    xs_d = nc.dram_tensor("xs", [128, BS * L], dt, kind="ExternalInput")
    mov_d = nc.dram_tensor("mov", [2, 128, NCOL], dt, kind="ExternalInput")
    cw3_d = nc.dram_tensor("cw3", [128, BS * P3], dt, kind="ExternalInput")
    w2s_d = nc.dram_tensor("w2s", [128, BS * P2], dt, kind="ExternalInput")
    w1s_d = nc.dram_tensor("w1s", [128, BS], dt, kind="ExternalInput")
    u1b_d = nc.dram_tensor("u1b", [128, L], dt, kind="ExternalInput")
    id_d = nc.dram_tensor("ident", [128, 128], dt, kind="ExternalInput")
    out_d = nc.dram_tensor("outT", [128, BS], dt, kind="ExternalOutput")

    mult = mybir.AluOpType.mult
    add = mybir.AluOpType.add

    with tile.TileContext(nc) as tc, ExitStack() as ctx:
        const = ctx.enter_context(tc.tile_pool(name="const", bufs=1))
        work = ctx.enter_context(tc.tile_pool(name="work", bufs=4))
        ps_xx = ctx.enter_context(
            tc.tile_pool(name="ps_xx", bufs=3, space=bass.MemorySpace.PSUM)
        )
        ps_t3 = ctx.enter_context(
            tc.tile_pool(name="ps_t3", bufs=5, space=bass.MemorySpace.PSUM)
        )

        XS = const.tile([128, BS * L], dt)
        nc.sync.dma_start(XS[:], xs_d.ap())
        MOV0 = const.tile([128, NCOL], dt)
        nc.sync.dma_start(MOV0[:], mov_d.ap()[0])
        MOV1 = const.tile([128, NCOL], dt)
        nc.sync.dma_start(MOV1[:], mov_d.ap()[1])
        CW3 = const.tile([128, BS * P3], dt)
        nc.sync.dma_start(CW3[:], cw3_d.ap())
        W2S = const.tile([128, BS * P2], dt)
        nc.sync.dma_start(W2S[:], w2s_d.ap())
        W1S = const.tile([128, BS], dt)
        nc.sync.dma_start(W1S[:], w1s_d.ap())
        U1B = const.tile([128, L], dt)
        nc.sync.dma_start(U1B[:], u1b_d.ap())
        IDENT = const.tile([128, 128], dt)
        nc.sync.dma_start(IDENT[:], id_d.ap())

        # xx[f, (n, m2, k)] = x[f, n, m2] * x[f, n, k] — split into quarters
        # so PE transposes start as soon as the first quarter is ready
        XX = const.tile([128, BS * L * L], dt)
        Q = 4
        for qi in range(Q):
            nq = BS // Q
            xs3 = XS[:, qi * nq * L : (qi + 1) * nq * L].rearrange(
                "p (n a) -> p n a", n=nq
            )
            in0 = xs3.unsqueeze(3).broadcast_to((128, nq, L, L))  # x_m2
            in1 = xs3.unsqueeze(2).broadcast_to((128, nq, L, L))  # x_k
            xxv = XX[:, qi * nq * 256 : (qi + 1) * nq * 256].rearrange(
                "p (n a b) -> p n a b", n=nq, a=L
            )
            nc.vector.tensor_tensor(xxv, in0, in1, op=mult)

        OUT = const.tile([128, BS], dt)
        ACC = const.tile([128, BS * 3], dt)  # per atom: [lin, quad, cubic]
        XXS = const.tile([128, BS * 256], dt)  # all atoms' transposed xx
        W3X = const.tile([128, BS * NPM], dt)  # all atoms' W3*x weights

        maybe_r = (
            (lambda ap: ap.bitcast(mybir.dt.float32r)) if use_f32r else (lambda ap: ap)
        )

        # Phase A: PE transposes + ACT evacuation; GPSIMD builds w3x in parallel
        for n in range(BS):
            xxp = ps_xx.tile([128, 256], dt, tag="xxp")
            nc.tensor.transpose(
                xxp[:, 0:128], XX[:, n * 256 : n * 256 + 128], IDENT[:]
            )
            nc.tensor.transpose(
                xxp[:, 128:256], XX[:, n * 256 + 128 : n * 256 + 256], IDENT[:]
            )
            nc.scalar.copy(XXS[:, n * 256 : (n + 1) * 256], xxp[:])

            # w3x[f, (p, m1)] = W3[e_n, p, f] * x[f, m1]  (GPSIMD, off DVE)
            c0 = CW3[:, n * P3 : (n + 1) * P3].unsqueeze(2).broadcast_to((128, P3, L))
            x0 = XS[:, n * L : (n + 1) * L].unsqueeze(1).broadcast_to((128, P3, L))
            nc.gpsimd.tensor_tensor(
                W3X[:, n * NPM : (n + 1) * NPM].rearrange(
                    "p (a b) -> p a b", a=P3
                ),
                c0,
                x0,
                op=mult,
            )
            # linear: sum_l x_l * W1 * U1_l (DVE, independent of PE)
            sc16 = work.tile([128, L], dt, tag="sc16")
            nc.vector.scalar_tensor_tensor(
                out=sc16[:],
                in0=XS[:, n * L : (n + 1) * L],
                scalar=W1S[:, n : n + 1],
                in1=U1B[:],
                op0=mult,
                op1=mult,
                accum_out=ACC[:, 3 * n : 3 * n + 1],
            )

        # Phase B: PE matmuls + DVE reductions chasing them
        for n in range(BS):
            t3 = ps_t3.tile([128, NCOL], dt, tag="t3")
            nc.tensor.matmul(
                t3[:],
                maybe_r(XXS[:, n * 256 : n * 256 + 128]),
                maybe_r(MOV0[:]),
                start=True,
                stop=False,
            )
            nc.tensor.matmul(
                t3[:],
                maybe_r(XXS[:, n * 256 + 128 : n * 256 + 256]),
                maybe_r(MOV1[:]),
                start=False,
                stop=True,
            )
            # quadratic: sum_q T2_q * W2_q
            sc4 = work.tile([128, P2], dt, tag="sc4")
            nc.vector.scalar_tensor_tensor(
                out=sc4[:],
                in0=t3[:, NPM:NCOL],
                scalar=1.0,
                in1=W2S[:, n * P2 : (n + 1) * P2],
                op0=mult,
                op1=mult,
                accum_out=ACC[:, 3 * n + 1 : 3 * n + 2],
            )
            # cubic: sum_{p,m1} T3 * w3x
            sc368 = work.tile([128, NPM], dt, tag="sc368")
            nc.vector.scalar_tensor_tensor(
                out=sc368[:],
                in0=t3[:, 0:NPM],
                scalar=1.0,
                in1=W3X[:, n * NPM : (n + 1) * NPM],
                op0=mult,
                op1=mult,
                accum_out=ACC[:, 3 * n + 2 : 3 * n + 3],
            )

        nc.vector.tensor_reduce(
            OUT[:],
            ACC[:].rearrange("p (n c) -> p n c", n=BS),
            axis=mybir.AxisListType.X,
            op=add,
        )
        nc.sync.dma_start(out_d.ap(), OUT[:])

    nc.compile()
    return nc


def _host_prep(x, sorted_indices, weights_max, w2, w1, U3, U2, U1):
    """Build per-core input maps (pure layout/gather work)."""
    x = np.ascontiguousarray(x, dtype=np.float32)
    si = np.asarray(sorted_indices).astype(np.int64)
    W3 = np.asarray(weights_max, dtype=np.float32)
    W2 = np.asarray(w2, dtype=np.float32)
    W1 = np.asarray(w1, dtype=np.float32)
    U3 = np.asarray(U3, dtype=np.float32)
    U2 = np.asarray(U2, dtype=np.float32)
    U1 = np.asarray(U1, dtype=np.float32)

    U3r = U3.reshape(P3, L, L, L)  # [p, m1, m2, k]
    U3m = np.ascontiguousarray(U3r.transpose(2, 3, 0, 1).reshape(L * L, P3 * L))
    U2m = np.ascontiguousarray(U2.reshape(P2, L * L).T)  # [(a,b), q]
    mov = np.concatenate([U3m, U2m], axis=1).reshape(2, 128, NCOL)
    mov = np.ascontiguousarray(mov)

    u1b = np.ascontiguousarray(np.tile(U1.reshape(1, L), (128, 1)))
    ident = np.eye(128, dtype=np.float32)

    in_maps = []
    for c in range(NCORES):
        sl = slice(c * BS, (c + 1) * BS)
        sic = si[sl]
        xs = np.ascontiguousarray(x[sl].transpose(1, 0, 2).reshape(128, BS * L))
        cw3 = np.ascontiguousarray(
            W3[sic].transpose(2, 0, 1).reshape(128, BS * P3)
        )
        w2s = np.ascontiguousarray(W2[sic].transpose(2, 0, 1).reshape(128, BS * P2))
        w1s = np.ascontiguousarray(W1[sic][:, 0, :].T)
        in_maps.append(
            {
                "xs": xs,
                "mov": mov,
                "cw3": cw3,
                "w2s": w2s,
                "w1s": w1s,
                "u1b": u1b,
                "ident": ident,
            }
        )
    return in_maps


def _get_nc():
    use_f32r = os.environ.get("KERNEL_F32R", "0") == "1"
    key = ("nc", use_f32r)
    if key not in _CACHE:
        _CACHE[key] = _build_program(use_f32r)
    return _CACHE[key]


def kernel(
    x,
    bincount,
    sorted_indices,
    weights_max,
    w2,
    w1,
    U3,
    U2,
    U1,
    _trace=False,
):
    from concourse.bass_utils import run_bass_kernel_spmd

    nc = _get_nc()
    in_maps = _host_prep(x, sorted_indices, weights_max, w2, w1, U3, U2, U1)
    res = run_bass_kernel_spmd(
        nc, in_maps, core_ids=list(range(NCORES)), trace=_trace
    )
    outs = [res.results[c]["outT"] for c in range(NCORES)]  # each [128f, 16n]
    full = np.concatenate([o.T for o in outs], axis=0)  # [128, 128]
    out = np.ascontiguousarray(full, dtype=np.float32)
    if _trace:
        return out, res
    return out
